# revision 2
# baseline (speedup 1.0000x reference)
"""Trainium2 Bass kernel for nn_Blur: depthwise 4x4 binomial blur.

Reference op: x (8, 64, 512, 512) fp32, pad (1,1,1,1), depthwise conv with
k2 = outer([1,3,3,1],[1,3,3,1])/64, stride 1 -> out (8, 64, 511, 511).

Strategy (pure data parallel, batch sharded across 8 cores):
  Each core processes one batch element = 64 images of 512x512.
  Per image, output rows are produced in 5 chunks (125,125,125,125,11 rows).
  The whole 2D blur for a chunk is 4 PSUM-accumulated matmuls:
      out[m, w] = sum_dx  Band_dx^T @ tile[:, dx : dx+512]
  where Band_dx[r, m] = kv[r-m] * kv[dx] / 64 is the banded vertical-blur
  matrix (stationary) and the moving operand is the horizontally shifted
  image tile. Horizontal/vertical padding is handled by zeroed border
  columns / a zeroed pad row + band row slicing.

  Compute dtype is float32r (PE fast fp32 mode, ~1 cycle/row, ~1e-4 rel
  error); inputs are rounded to f32r during the SWDGE input DMA. PSUM is
  fp32; ScalarE evacuates PSUM -> SBUF; HWDGE stores fp32 results.
"""
import os
import numpy as np

import bass_rust
import concourse.tile as tile
from concourse import mybir, bass_utils, bacc
from contextlib import ExitStack

B, C, H, W = 8, 64, 512, 512
HO = WO = 511
N_CORES = 8
NCHUNK = 5  # output row chunks per image: 4 x 125 + 1 x 11
M_MAIN, M_LAST = 125, 11
K_LAST = 13
TW = 516  # padded tile width: 1 left zero col + 512 img cols + 3 right zero cols
NMM = 512  # matmul moving free size (f32r requires even N); out col 511 discarded

# Results of the last traced run (populated when BLUR_TRACE=1), for test.py.
LAST_EXEC_TIME_NS = None
LAST_SCOPE_TIMES = None

_cached = None


def _make_bands() -> np.ndarray:
    kv = np.array([1.0, 3.0, 3.0, 1.0], np.float32)
    bands = np.zeros((128, 4, M_MAIN), np.float32)
    for dx in range(4):
        for m in range(M_MAIN):
            for d in range(4):
                bands[m + d, dx, m] = kv[d] * kv[dx] / 64.0
    return bands


def _custom_ap(base_ap, dims, offset):
    """AP with explicit [(stride, size), ...] dims and element offset."""
    ap = base_ap.copy()
    ap.ap = bass_rust.VecI64Pair(dims)
    ap.offset = offset
    return ap


def _build_program():
    nc = bacc.Bacc("TRN2", target_bir_lowering=False, debug=False, num_devices=1)
    x_d = nc.dram_tensor("x", [C, H, W], mybir.dt.float32, kind="ExternalInput")
    b_d = nc.dram_tensor("bands", [128, 4, M_MAIN], mybir.dt.float32, kind="ExternalInput")
    o_d = nc.dram_tensor("out", [C, HO, WO], mybir.dt.float32, kind="ExternalOutput")
    x_ap = x_d.ap()
    o_ap = o_d.ap()

    with tile.TileContext(nc) as tc:
        with ExitStack() as ctx:
            inp = ctx.enter_context(tc.tile_pool(name="inp", bufs=4))
            stg = ctx.enter_context(tc.tile_pool(name="stg", bufs=4))
            cst = ctx.enter_context(tc.tile_pool(name="cst", bufs=1))
            pp = ctx.enter_context(tc.tile_pool(name="pp", bufs=6, space="PSUM"))

            bands = cst.tile([128, 4, M_MAIN], mybir.dt.float32r)
            nc.gpsimd.dma_start(bands[:], b_d.ap())

            for img in range(C):
                t = inp.tile([128, NCHUNK, TW], mybir.dt.float32r, tag="t")
                # zero borders: left col, right 3 cols of each chunk, pad row
                nc.vector.memset(t[:, :, 0].bitcast(mybir.dt.float32), 0.0)
                nc.vector.memset(t[:, :, 513:516].bitcast(mybir.dt.float32), 0.0)
                nc.vector.memset(t[0:1, 0, :].bitcast(mybir.dt.float32), 0.0)
                # input loads (SWDGE, fp32 -> f32r rounding in-flight)
                nc.gpsimd.dma_start(t[1:128, 0, 1:513], x_ap[img, 0:127, :])
                interior = _custom_ap(
                    x_ap[img],
                    [(W, 128), (M_MAIN * W, 3), (1, W)],
                    img * H * W + (M_MAIN - 1) * W,
                )
                nc.gpsimd.dma_start(t[0:128, 1:4, 1:513], interior)
                nc.gpsimd.dma_start(t[0:K_LAST, 4, 1:513], x_ap[img, 499:512, :])

                st = stg.tile([128, NCHUNK, NMM], mybir.dt.float32, tag="st")
                for c in range(NCHUNK):
                    kk = 128 if c < 4 else K_LAST
                    mm = M_MAIN if c < 4 else M_LAST
                    pt = pp.tile([128, NMM], mybir.dt.float32, tag="pt")
                    for dx in range(4):
                        nc.tensor.matmul(
                            pt[0:mm, :],
                            bands[0:kk, dx, 0:mm],
                            t[0:kk, c, dx : dx + NMM],
                            start=(dx == 0),
                            stop=(dx == 3),
                        )
                    nc.scalar.copy(st[0:mm, c, :], pt[0:mm, :])

                out_main = _custom_ap(
                    o_ap[img],
                    [(WO, M_MAIN), (M_MAIN * WO, 4), (1, WO)],
                    img * HO * WO,
                )
                nc.sync.dma_start(out_main, st[0:M_MAIN, 0:4, 0:WO])
                nc.sync.dma_start(o_ap[img, 500:511, :], st[0:M_LAST, 4, 0:WO])

    nc.compile()
    return nc


def kernel(x: np.ndarray) -> np.ndarray:
    global _cached, LAST_EXEC_TIME_NS, LAST_SCOPE_TIMES
    assert x.shape == (B, C, H, W), x.shape
    if _cached is None:
        _cached = _build_program()
    nc = _cached

    bands = _make_bands()
    x = np.ascontiguousarray(x, dtype=np.float32)
    in_maps = [{"x": x[core], "bands": bands} for core in range(N_CORES)]

    trace = os.environ.get("BLUR_TRACE", "0") == "1"
    kwargs = {}
    if trace:
        kwargs = dict(trace=True, stitch_traces=False)
        td = os.environ.get("BLUR_TRACE_DIR")
        if td:
            kwargs["tmpdir"] = td
    res = bass_utils.run_bass_kernel_spmd(
        nc, in_maps, core_ids=list(range(N_CORES)), **kwargs
    )
    if trace:
        LAST_EXEC_TIME_NS = res.exec_time_ns
        LAST_SCOPE_TIMES = res.per_core_scope_times

    out = np.stack([res.results[core]["out"] for core in range(N_CORES)])
    return out
